# revision 55
# baseline (speedup 1.0000x reference)
"""GCN layer (gather -> mean-aggregate -> linear) on 8 Trainium2 cores.

Strategy (SPMD, no collectives):
  - Nodes are row-sharded: core c owns dst nodes [c*S, (c+1)*S), S = N/8.
  - Edges are bucketed by dst-owner core on the host and turned into a dense
    per-core adjacency count matrix A[src_node, local_dst] (fp8e4m3 - counts
    are small integers, exact). The per-core segment sum is computed
    FEATURE-MAJOR on the PE array:  sumsT = x.T @ A, with x as bf16 slabs
    (lhsT stationary) and A streaming as the rhs.  This streams S columns
    per src slab (vs 2*F+1 per slab per node-group for the node-major
    orientation) - half the PE cycles of the hi/lo node-major scheme.
  - Degrees (and the zero-in-degree fallback) are folded in on the host:
    rb = 1/max(deg,1) is shipped per node, and zero-degree nodes get a
    self-edge in A so mean==x for them (DGL recv semantics), making the
    device program branch-free.
  - Phase 1 runs in two column passes (A-left cols 0:1024, A-right 1024:S,
    shipped as separate streams).  Pass A's eight node tiles drain under
    the pass-B matmul stream: per tile, evac PSUM->SBUF bf16 (sumsT is
    already the GEMM's lhsT layout - no PE transposes), GEMM + rank-1
    bias matmul into PSUM, a single per-partition scale to SBUF
    (DVE / ACT alternating), and the output DMA.
  - Input DMA chunks alternate between the two HWDGE queues in
    consumption order, with tiny head chunks so the PE starts early and
    the HAM clock never re-throttles.
"""

import os

import numpy as np

CORES = 8
TRACE = False           # set by test harness to print HW exec time
_cache = {}


def _build_program(N, F, FO, R, S):
    from concourse import bacc, tile
    from concourse.bass import mybir

    F32 = mybir.dt.float32
    BF16 = mybir.dt.bfloat16
    FP8 = mybir.dt.float8e4
    KT = (N + 127) // 128          # src-node slabs
    NT = (S + 127) // 128          # owned-node tiles per core
    CL = 1024                      # pass-A columns (8 node tiles)
    CR = S - CL                    # pass-B columns
    nc = bacc.Bacc(None)

    xtd = nc.dram_tensor("xt", [128, KT * F], BF16, kind="ExternalInput")
    Ad = nc.dram_tensor("A", [128, KT * S], FP8, kind="ExternalInput")
    Wd = nc.dram_tensor("W", [F, FO], BF16, kind="ExternalInput")
    bd = nc.dram_tensor("b", [128, FO], BF16, kind="ExternalInput")
    rbd = nc.dram_tensor("rb", [128, NT], F32, kind="ExternalInput")
    dbd = nc.dram_tensor("db", [128, S], BF16, kind="ExternalInput")
    out = nc.dram_tensor("out", [R, FO], F32, kind="ExternalOutput")

    # PSUM bank map (each accumulating group owns a 2KB bank, zeroed by
    # its first start=True matmul):
    #   banks 0,1: pass-A col groups [0:512],[512:1024]
    #   bank  2:   pass-B col group [1024:S]
    #   banks 3..6: drain out psum, cycling t%4
    #   bank  7 head: PE warm-up scratch
    psall = nc.alloc_psum_tensor("psall", [128, 4096], F32)

    with tile.TileContext(nc) as tc:
        with (
            tc.tile_pool(name="const", bufs=1) as cpool,
            tc.tile_pool(name="acc", bufs=1) as accpool,
            tc.tile_pool(name="p3", bufs=10) as p3pool,
        ):
            x_sb = accpool.tile([128, KT, F], BF16, name="x_sb", tag="x_sb")
            x_flat = x_sb[:].rearrange("p a b -> p (a b)")
            A_sb = accpool.tile([128, KT, S], FP8, name="A_sb", tag="A_sb")
            A_flat = A_sb[:].rearrange("p a b -> p (a b)")
            sumsT = accpool.tile([128, S], BF16, name="sumsT")
            wt_sb = cpool.tile([128, FO], BF16, name="wt_sb")
            b_sb = cpool.tile([128, FO], BF16, name="b_sb")
            rb_sb = cpool.tile([128, NT], F32, name="rb_sb")
            db_sb = cpool.tile([128, S], BF16, name="db_sb")

            # ---- all input DMA up front on two HWDGE rings, each ring's
            # backlog in consumption order.  A-left chunks alternate
            # between the rings; x chunks slot between them on the scalar
            # ring; constants are only needed at drain time and go LAST ----
            def dAl(k0, k1, eng):
                eng.dma_start(A_flat[:, k0 * S : k1 * S],
                              Ad[:, k0 * S : k1 * S])

            def dx(k0, k1, eng):
                eng.dma_start(x_flat[:, k0 * F : k1 * F],
                              xtd[:, k0 * F : k1 * F])

            xfers = []   # (order, kind, k0, k1)
            bal = [0, 1, 2, 4, 6, 9, 13, 17, 22, 27, 33, 39, 45, 51, 57,
                   63, 71, KT]
            for k0, k1 in zip(bal, bal[1:]):
                xfers.append(((k0, 1), "Al", k0, k1))
            bx = [0, 1, 2, 4, 8, 16, 32, 56, KT]
            for k0, k1 in zip(bx, bx[1:]):
                xfers.append(((k0, 0), "x", k0, k1))
            xfers.sort(key=lambda t: t[0])
            qi = 0
            for _, kind, k0, k1 in xfers:
                deng = nc.sync if qi % 2 == 0 else nc.scalar
                qi += 1
                if kind == "Al":
                    dAl(k0, k1, deng)
                else:
                    dx(k0, k1, deng)
                if qi == 16:
                    # drain-time constants: mid-stream, so they arrive well
                    # before the drains but steal no ramp bandwidth
                    nc.sync.dma_start(wt_sb[:], Wd[:])
                    nc.scalar.dma_start(b_sb[:], bd[:])
                    nc.sync.dma_start(rb_sb[:], rbd[:])
                    nc.scalar.dma_start(db_sb[:], dbd[:])

            # PE warm-up: tiny matmuls during the first-chunk DMA wait so
            # the HAM clock gate is at full rate for the real stream.
            warm = cpool.tile([128, 128], BF16, name="warm")
            nc.vector.memset(warm[:], 0.0)
            for _w in range(20):
                nc.tensor.matmul(
                    psall[:16, 3584:3712], warm[:, 0:16], warm[:, 0:128],
                    start=True, stop=True, skip_group_check=True,
                )

            # ---- phase 1: sumsT[f, d] += x[s, f] * A[s, d], one pass,
            # three bank groups per slab sharing one weight load ----
            for k in range(KT):
                st = k == 0
                sp = k == KT - 1
                for (c0, c1) in [(0, 512), (512, 1024), (1024, S)]:
                    nc.tensor.matmul(
                        psall[:, c0:c1], x_sb[:, k, :], A_sb[:, k, c0:c1],
                        start=st, stop=sp, skip_group_check=False,
                    )

            # evacuate (PSUM -> SBUF bf16), alternating engines
            for t in range(NT):
                cs = slice(128 * t, min(128 * (t + 1), S))
                if t % 2:
                    nc.scalar.copy(sumsT[:, cs], psall[:, cs])
                else:
                    nc.vector.tensor_scalar_mul(sumsT[:, cs], psall[:, cs], 1.0)

            # drain every node tile: GEMM (+rank-1 bias), scale, store
            for t in range(NT):
                m = min(128, S - 128 * t)      # last tile is a remnant
                rows = slice(128 * t, 128 * t + m)
                ps3 = psall[:, 1536 + (t % 4) * 512 : 2048 + (t % 4) * 512]
                ot = p3pool.tile([128, FO], F32, tag="ot")
                if t % 2:
                    nc.tensor.matmul(ps3[:m, :], sumsT[:, rows], wt_sb[:],
                                     start=True, stop=True,
                                     skip_group_check=True)
                    nc.vector.scalar_tensor_tensor(
                        ot[:m, :], ps3[:m, :], rb_sb[:m, t : t + 1], b_sb[:m, :],
                        op0=mybir.AluOpType.mult, op1=mybir.AluOpType.add,
                    )
                else:
                    # bias via rank-1 matmul into the same PSUM group: add
                    # b*deg pre-scale, the *rb evac restores b.
                    nc.tensor.matmul(ps3[:m, :], sumsT[:, rows], wt_sb[:],
                                     start=True, stop=False,
                                     skip_group_check=True)
                    nc.tensor.matmul(ps3[:m, :], db_sb[:, 128 * t : 128 * t + m],
                                     b_sb[:], start=False, stop=True,
                                     skip_group_check=True)
                    nc.scalar.mul(ot[:m, :], ps3[:m, :], rb_sb[:m, t : t + 1])
                deng = nc.scalar if t % 2 else nc.sync
                deng.dma_start(out[rows, :], ot[:m, :])

    nc.compile()
    return nc


def _shard_inputs(x32, src, dst, W32, b32, n_cores):
    import ml_dtypes

    N, F = x32.shape
    FO = W32.shape[1]
    S = (N + n_cores - 1) // n_cores
    NT = (S + 127) // 128
    R = NT * 128
    KT = (N + 127) // 128
    CL = 1024

    # x slabs, feature-minor: xt[p, k, f] = x[128k + p, f], bf16
    xp = np.zeros((KT * 128, F), np.float32)
    xp[:N] = x32
    xt = np.ascontiguousarray(
        xp.reshape(KT, 128, F).transpose(1, 0, 2).astype(ml_dtypes.bfloat16)
    ).reshape(128, KT * F)

    deg = np.bincount(dst, minlength=N)
    rb_full = (1.0 / np.maximum(deg, 1)).astype(np.float32)
    zero_nodes = np.where(deg == 0)[0]

    brep = np.ascontiguousarray(
        np.tile(b32.reshape(1, -1), (128, 1)).astype(ml_dtypes.bfloat16))
    Wb = W32.astype(ml_dtypes.bfloat16)

    in_maps = []
    for c in range(n_cores):
        lo = c * S
        hi = min(N, lo + S)
        sel = (dst >= lo) & (dst < hi)
        A = np.zeros((KT * 128, S), np.float32)
        np.add.at(A, (src[sel], dst[sel] - lo), 1.0)
        zn = zero_nodes[(zero_nodes >= lo) & (zero_nodes < hi)]
        if zn.size:  # self-edge: zero-in-degree nodes keep their input
            A[zn, zn - lo] += 1.0
        assert A.max() <= 16, "edge multiplicity too large for fp8e4m3"
        A8 = np.ascontiguousarray(
            A.reshape(KT, 128, S).transpose(1, 0, 2).astype(ml_dtypes.float8_e4m3)
        ).reshape(128, KT * S)
        rb_c = np.ones(R, np.float32)
        rb_c[: hi - lo] = rb_full[lo:hi]
        # db[n] = max(deg,1)/128 so (sums@W + db*128*b) * rb == mean@W + b
        deg_c = np.ones(S, np.float32)
        deg_c[: hi - lo] = np.maximum(deg[lo:hi], 1)
        db_c = np.ascontiguousarray(np.tile(
            (deg_c / 128.0).astype(ml_dtypes.bfloat16).reshape(1, S),
            (128, 1)))
        rb_c = np.ascontiguousarray(rb_c.reshape(NT, 128).T)
        in_maps.append({"xt": xt, "A": A8, "W": Wb, "b": brep,
                        "rb": rb_c, "db": db_c})
    return in_maps, R


def _install_ntff_shim():
    """antenv.axon_hooks shim so trace=True can NTFF-profile in this env."""
    import contextlib
    import ctypes
    import sys
    import types

    if "antenv.axon_hooks" in sys.modules:
        return
    so_path = "/opt/axon/libaxon_pjrt.so"
    try:
        lib = ctypes.CDLL(so_path)
        lib.axon_start_nrt_profile.argtypes = [
            ctypes.POINTER(ctypes.c_int64), ctypes.c_size_t]
        lib.axon_start_nrt_profile.restype = ctypes.c_int64
        lib.axon_stop_nrt_profile.argtypes = [ctypes.c_char_p]
        lib.axon_stop_nrt_profile.restype = ctypes.c_int64
    except Exception:
        return

    @contextlib.contextmanager
    def _hook(output_dir, device_ids):
        import jax

        jax.devices()
        if device_ids:
            ids = (ctypes.c_int64 * len(device_ids))(*device_ids)
            rc = lib.axon_start_nrt_profile(ids, len(device_ids))
        else:
            rc = lib.axon_start_nrt_profile(None, 0)
        if rc != 0:
            raise RuntimeError(f"axon_start_nrt_profile rc={rc}")
        try:
            yield
        finally:
            lib.axon_stop_nrt_profile(str(output_dir).encode())

    mod = types.ModuleType("antenv.axon_hooks")
    mod.set_axon_ntff_profile_hook = lambda h: None
    mod.get_axon_ntff_profile_hook = lambda: _hook
    sys.modules["antenv.axon_hooks"] = mod


def kernel(x, src, dst, W, b):
    from concourse import bass_utils

    x32 = np.ascontiguousarray(np.asarray(x), dtype=np.float32)
    W32 = np.ascontiguousarray(np.asarray(W), dtype=np.float32)
    b32 = np.ascontiguousarray(np.asarray(b), dtype=np.float32)
    src = np.asarray(src).astype(np.int64)
    dst = np.asarray(dst).astype(np.int64)
    N, F = x32.shape
    FO = W32.shape[1]
    S = (N + CORES - 1) // CORES

    in_maps, R = _shard_inputs(x32, src, dst, W32, b32, CORES)

    key = (N, F, FO, R)
    if key not in _cache:
        _cache[key] = _build_program(N, F, FO, R, S)
    nc = _cache[key]

    if TRACE:
        _install_ntff_shim()

    last_err = None
    for _attempt in range(2):
        try:
            res = bass_utils.run_bass_kernel_spmd(
                nc, in_maps, core_ids=list(range(CORES)), trace=TRACE
            )
            break
        except Exception as e:  # retry once on transient device errors
            last_err = e
    else:
        raise last_err

    if TRACE and res.exec_time_ns is not None:
        print("HW exec time:", res.exec_time_ns, "ns")

    outs = [np.asarray(r["out"]).reshape(R, FO) for r in res.results]
    full = np.concatenate([o[:S] for o in outs], axis=0)[:N]
    return full.astype(np.float32)


# revision 56
# speedup vs baseline: 1.1113x; 1.1113x over previous
"""GCN layer (gather -> mean-aggregate -> linear) on 8 Trainium2 cores.

Strategy (SPMD, no collectives):
  - Nodes are row-sharded: core c owns dst nodes [c*S, (c+1)*S), S = N/8.
  - Edges are bucketed by dst-owner core on the host and turned into a dense
    per-core adjacency count matrix A[src_node, local_dst] (fp8e4m3 - counts
    are small integers, exact). The per-core segment sum is computed
    FEATURE-MAJOR on the PE array:  sumsT = x.T @ A, with x as bf16 slabs
    (lhsT stationary) and A streaming as the rhs.  This streams S columns
    per src slab (vs 2*F+1 per slab per node-group for the node-major
    orientation) - half the PE cycles of the hi/lo node-major scheme.
  - Degrees (and the zero-in-degree fallback) are folded in on the host:
    rb = 1/max(deg,1) is shipped per node, and zero-degree nodes get a
    self-edge in A so mean==x for them (DGL recv semantics), making the
    device program branch-free.
  - Phase 1 runs in two column passes (A-left cols 0:1024, A-right 1024:S,
    shipped as separate streams).  Pass A's eight node tiles drain under
    the pass-B matmul stream: per tile, evac PSUM->SBUF bf16 (sumsT is
    already the GEMM's lhsT layout - no PE transposes), GEMM + rank-1
    bias matmul into PSUM, a single per-partition scale to SBUF
    (DVE / ACT alternating), and the output DMA.
  - Input DMA chunks alternate between the two HWDGE queues in
    consumption order, with tiny head chunks so the PE starts early and
    the HAM clock never re-throttles.
"""

import os

import numpy as np

CORES = 8
TRACE = False           # set by test harness to print HW exec time
_cache = {}


def _build_program(N, F, FO, R, S):
    from concourse import bacc, tile
    from concourse.bass import mybir

    F32 = mybir.dt.float32
    BF16 = mybir.dt.bfloat16
    FP8 = mybir.dt.float8e4
    KT = (N + 127) // 128          # src-node slabs
    NT = (S + 127) // 128          # owned-node tiles per core
    CL = 1024                      # pass-A columns (8 node tiles)
    CR = S - CL                    # pass-B columns
    nc = bacc.Bacc(None)

    xtd = nc.dram_tensor("xt", [128, KT * F], BF16, kind="ExternalInput")
    Ald = nc.dram_tensor("Al", [128, KT * CL], FP8, kind="ExternalInput")
    Ard = nc.dram_tensor("Ar", [128, KT * CR], FP8, kind="ExternalInput")
    Wd = nc.dram_tensor("W", [F, FO], BF16, kind="ExternalInput")
    bd = nc.dram_tensor("b", [128, FO], BF16, kind="ExternalInput")
    rbd = nc.dram_tensor("rb", [128, NT], F32, kind="ExternalInput")
    dbd = nc.dram_tensor("db", [128, S], BF16, kind="ExternalInput")
    out = nc.dram_tensor("out", [R, FO], F32, kind="ExternalOutput")

    # PSUM bank map (each accumulating group owns a 2KB bank, zeroed by
    # its first start=True matmul):
    #   banks 0,1: pass-A col groups [0:512],[512:1024]
    #   bank  2:   pass-B col group [1024:S]
    #   banks 3..6: drain out psum, cycling t%4
    #   bank  7 head: PE warm-up scratch
    psall = nc.alloc_psum_tensor("psall", [128, 4096], F32)

    with tile.TileContext(nc) as tc:
        with (
            tc.tile_pool(name="const", bufs=1) as cpool,
            tc.tile_pool(name="acc", bufs=1) as accpool,
            tc.tile_pool(name="p3", bufs=10) as p3pool,
        ):
            x_sb = accpool.tile([128, KT, F], BF16, name="x_sb", tag="x_sb")
            x_flat = x_sb[:].rearrange("p a b -> p (a b)")
            Al_sb = accpool.tile([128, KT, CL], FP8, name="Al_sb", tag="Al_sb")
            Al_flat = Al_sb[:].rearrange("p a b -> p (a b)")
            Ar_sb = accpool.tile([128, KT, CR], FP8, name="Ar_sb", tag="Ar_sb")
            Ar_flat = Ar_sb[:].rearrange("p a b -> p (a b)")
            sumsT = accpool.tile([128, S], BF16, name="sumsT")
            wt_sb = cpool.tile([128, FO], BF16, name="wt_sb")
            b_sb = cpool.tile([128, FO], BF16, name="b_sb")
            rb_sb = cpool.tile([128, NT], F32, name="rb_sb")
            db_sb = cpool.tile([128, S], BF16, name="db_sb")

            # ---- all input DMA up front on two HWDGE rings, each ring's
            # backlog in consumption order.  A-left chunks alternate
            # between the rings; x chunks slot between them on the scalar
            # ring; constants are only needed at drain time and go LAST ----
            def dAl(k0, k1, eng):
                eng.dma_start(Al_flat[:, k0 * CL : k1 * CL],
                              Ald[:, k0 * CL : k1 * CL])

            def dx(k0, k1, eng):
                eng.dma_start(x_flat[:, k0 * F : k1 * F],
                              xtd[:, k0 * F : k1 * F])

            xfers = []   # (order, kind, k0, k1)
            bal = [0, 1, 2, 4, 6, 9, 13, 17, 22, 27, 33, 39, 45, 51, 57,
                   63, 71, KT]
            for k0, k1 in zip(bal, bal[1:]):
                xfers.append(((k0, 1), "Al", k0, k1))
            bx = [0, 1, 2, 4, 8, 16, 32, 56, KT]
            for k0, k1 in zip(bx, bx[1:]):
                xfers.append(((k0, 0), "x", k0, k1))
            xfers.sort(key=lambda t: t[0])
            qi = 0
            for _, kind, k0, k1 in xfers:
                deng = nc.sync if qi % 2 == 0 else nc.scalar
                qi += 1
                if kind == "Al":
                    dAl(k0, k1, deng)
                else:
                    dx(k0, k1, deng)
                if qi == 16:
                    # drain-time constants: mid-stream, so they arrive well
                    # before the drains but steal no ramp bandwidth
                    nc.sync.dma_start(wt_sb[:], Wd[:])
                    nc.scalar.dma_start(b_sb[:], bd[:])
                    nc.sync.dma_start(rb_sb[:], rbd[:])
                    nc.scalar.dma_start(db_sb[:], dbd[:])
            nc.sync.dma_start(Ar_flat[:, : 40 * CR], Ard[:, : 40 * CR])
            nc.scalar.dma_start(Ar_flat[:, 40 * CR :], Ard[:, 40 * CR :])

            # PE warm-up: tiny matmuls during the first-chunk DMA wait so
            # the HAM clock gate is at full rate for the real stream.
            warm = cpool.tile([128, 128], BF16, name="warm")
            nc.vector.memset(warm[:], 0.0)
            for _w in range(20):
                nc.tensor.matmul(
                    psall[:16, 3584:3712], warm[:, 0:16], warm[:, 0:128],
                    start=True, stop=True, skip_group_check=True,
                )

            # ---- phase 1 pass A: sumsT[f, d] += x[s, f] * A[s, d] ----
            for k in range(KT):
                st = k == 0
                sp = k == KT - 1
                for (c0, c1) in [(0, 512), (512, 1024)]:
                    nc.tensor.matmul(
                        psall[:, c0:c1], x_sb[:, k, :], Al_sb[:, k, c0:c1],
                        start=st, stop=sp, skip_group_check=False,
                    )

            # evacuate pass-A tiles (PSUM -> SBUF bf16), alternating engines
            for t in range(8):
                cs = slice(128 * t, 128 * (t + 1))
                if t % 2:
                    nc.scalar.copy(sumsT[:, cs], psall[:, cs])
                else:
                    nc.vector.tensor_scalar_mul(sumsT[:, cs], psall[:, cs], 1.0)

            # drain one node tile: GEMM (+rank-1 bias), scale, store
            ots = []

            def drain_tile(t):
                m = min(128, S - 128 * t)      # last tile is a remnant
                rows = slice(128 * t, 128 * t + m)
                ps3 = psall[:, 1536 + (t % 4) * 512 : 2048 + (t % 4) * 512]
                ot = p3pool.tile([128, FO], F32, tag="ot")
                if t % 2:
                    nc.tensor.matmul(ps3[:m, :], sumsT[:, rows], wt_sb[:],
                                     start=True, stop=True,
                                     skip_group_check=True)
                    nc.vector.scalar_tensor_tensor(
                        ot[:m, :], ps3[:m, :], rb_sb[:m, t : t + 1], b_sb[:m, :],
                        op0=mybir.AluOpType.mult, op1=mybir.AluOpType.add,
                    )
                else:
                    # bias via rank-1 matmul into the same PSUM group: add
                    # b*deg pre-scale, the *rb evac restores b.
                    nc.tensor.matmul(ps3[:m, :], sumsT[:, rows], wt_sb[:],
                                     start=True, stop=False,
                                     skip_group_check=True)
                    nc.tensor.matmul(ps3[:m, :], db_sb[:, 128 * t : 128 * t + m],
                                     b_sb[:], start=False, stop=True,
                                     skip_group_check=True)
                    nc.scalar.mul(ot[:m, :], ps3[:m, :], rb_sb[:m, t : t + 1])
                deng = nc.scalar if t % 2 else nc.sync
                deng.dma_start(out[rows, :], ot[:m, :])

            # ---- phase 1 pass B, pass-A tiles draining underneath ----
            nxt_drain = 0
            for k in range(KT):
                st = k == 0
                sp = k == KT - 1
                nc.tensor.matmul(
                    psall[:, 1024 : 1024 + CR], x_sb[:, k, :], Ar_sb[:, k, :],
                    start=st, stop=sp, skip_group_check=False,
                )
                if k % 4 == 3 and nxt_drain < 8:
                    drain_tile(nxt_drain)
                    nxt_drain += 1

            # evacuate + drain the pass-B tiles
            for t in range(8, NT):
                cs = slice(128 * t, min(128 * (t + 1), S))
                if t % 2:
                    nc.scalar.copy(sumsT[:, cs], psall[:, cs])
                else:
                    nc.vector.tensor_scalar_mul(sumsT[:, cs], psall[:, cs], 1.0)
            while nxt_drain < NT:
                drain_tile(nxt_drain)
                nxt_drain += 1

    nc.compile()
    return nc


def _shard_inputs(x32, src, dst, W32, b32, n_cores):
    import ml_dtypes

    N, F = x32.shape
    FO = W32.shape[1]
    S = (N + n_cores - 1) // n_cores
    NT = (S + 127) // 128
    R = NT * 128
    KT = (N + 127) // 128
    CL = 1024

    # x slabs, feature-minor: xt[p, k, f] = x[128k + p, f], bf16
    xp = np.zeros((KT * 128, F), np.float32)
    xp[:N] = x32
    xt = np.ascontiguousarray(
        xp.reshape(KT, 128, F).transpose(1, 0, 2).astype(ml_dtypes.bfloat16)
    ).reshape(128, KT * F)

    deg = np.bincount(dst, minlength=N)
    rb_full = (1.0 / np.maximum(deg, 1)).astype(np.float32)
    zero_nodes = np.where(deg == 0)[0]

    brep = np.ascontiguousarray(
        np.tile(b32.reshape(1, -1), (128, 1)).astype(ml_dtypes.bfloat16))
    Wb = W32.astype(ml_dtypes.bfloat16)

    in_maps = []
    for c in range(n_cores):
        lo = c * S
        hi = min(N, lo + S)
        sel = (dst >= lo) & (dst < hi)
        A = np.zeros((KT * 128, S), np.float32)
        np.add.at(A, (src[sel], dst[sel] - lo), 1.0)
        zn = zero_nodes[(zero_nodes >= lo) & (zero_nodes < hi)]
        if zn.size:  # self-edge: zero-in-degree nodes keep their input
            A[zn, zn - lo] += 1.0
        assert A.max() <= 16, "edge multiplicity too large for fp8e4m3"
        A3 = A.reshape(KT, 128, S).transpose(1, 0, 2).astype(ml_dtypes.float8_e4m3)
        Al = np.ascontiguousarray(A3[:, :, :CL]).reshape(128, KT * CL)
        Ar = np.ascontiguousarray(A3[:, :, CL:]).reshape(128, KT * (S - CL))
        rb_c = np.ones(R, np.float32)
        rb_c[: hi - lo] = rb_full[lo:hi]
        # db[n] = max(deg,1)/128 so (sums@W + db*128*b) * rb == mean@W + b
        deg_c = np.ones(S, np.float32)
        deg_c[: hi - lo] = np.maximum(deg[lo:hi], 1)
        db_c = np.ascontiguousarray(np.tile(
            (deg_c / 128.0).astype(ml_dtypes.bfloat16).reshape(1, S),
            (128, 1)))
        rb_c = np.ascontiguousarray(rb_c.reshape(NT, 128).T)
        in_maps.append({"xt": xt, "Al": Al, "Ar": Ar, "W": Wb, "b": brep,
                        "rb": rb_c, "db": db_c})
    return in_maps, R


def _install_ntff_shim():
    """antenv.axon_hooks shim so trace=True can NTFF-profile in this env."""
    import contextlib
    import ctypes
    import sys
    import types

    if "antenv.axon_hooks" in sys.modules:
        return
    so_path = "/opt/axon/libaxon_pjrt.so"
    try:
        lib = ctypes.CDLL(so_path)
        lib.axon_start_nrt_profile.argtypes = [
            ctypes.POINTER(ctypes.c_int64), ctypes.c_size_t]
        lib.axon_start_nrt_profile.restype = ctypes.c_int64
        lib.axon_stop_nrt_profile.argtypes = [ctypes.c_char_p]
        lib.axon_stop_nrt_profile.restype = ctypes.c_int64
    except Exception:
        return

    @contextlib.contextmanager
    def _hook(output_dir, device_ids):
        import jax

        jax.devices()
        if device_ids:
            ids = (ctypes.c_int64 * len(device_ids))(*device_ids)
            rc = lib.axon_start_nrt_profile(ids, len(device_ids))
        else:
            rc = lib.axon_start_nrt_profile(None, 0)
        if rc != 0:
            raise RuntimeError(f"axon_start_nrt_profile rc={rc}")
        try:
            yield
        finally:
            lib.axon_stop_nrt_profile(str(output_dir).encode())

    mod = types.ModuleType("antenv.axon_hooks")
    mod.set_axon_ntff_profile_hook = lambda h: None
    mod.get_axon_ntff_profile_hook = lambda: _hook
    sys.modules["antenv.axon_hooks"] = mod


def kernel(x, src, dst, W, b):
    from concourse import bass_utils

    x32 = np.ascontiguousarray(np.asarray(x), dtype=np.float32)
    W32 = np.ascontiguousarray(np.asarray(W), dtype=np.float32)
    b32 = np.ascontiguousarray(np.asarray(b), dtype=np.float32)
    src = np.asarray(src).astype(np.int64)
    dst = np.asarray(dst).astype(np.int64)
    N, F = x32.shape
    FO = W32.shape[1]
    S = (N + CORES - 1) // CORES

    in_maps, R = _shard_inputs(x32, src, dst, W32, b32, CORES)

    key = (N, F, FO, R)
    if key not in _cache:
        _cache[key] = _build_program(N, F, FO, R, S)
    nc = _cache[key]

    if TRACE:
        _install_ntff_shim()

    last_err = None
    for _attempt in range(2):
        try:
            res = bass_utils.run_bass_kernel_spmd(
                nc, in_maps, core_ids=list(range(CORES)), trace=TRACE
            )
            break
        except Exception as e:  # retry once on transient device errors
            last_err = e
    else:
        raise last_err

    if TRACE and res.exec_time_ns is not None:
        print("HW exec time:", res.exec_time_ns, "ns")

    outs = [np.asarray(r["out"]).reshape(R, FO) for r in res.results]
    full = np.concatenate([o[:S] for o in outs], axis=0)[:N]
    return full.astype(np.float32)
